# revision 1
# baseline (speedup 1.0000x reference)
"""AdaptiveMLP Trainium2 kernel (8-core data parallel), v2.

Math per layer: y[b,o] = sum_{n,i} co[b,n]*x[b,i]*W[n,i,o] + sum_n co[b,n]*b[n,o]

Feature-major chain per core (B_LOC=8192 samples; column c <-> sample
b = (c%128)*64 + c//128):
  - u0coT [42, B]: rows n*3+i = co_n*x_i, rows 32+n = co_n (DVE mult of
    host-prepared xrep/corep).
  - L0: one matmul per 512-chunk with stationary [W0flat | W0flat]
    [42, 128] -> psum [128, 512] holds z1 TWICE (rows 0:64 == 64:128);
    single Act relu -> x1dup [128, B] (duplication for free, no DMA).
  - L1 per 2048-block: xp_p = x1dup * cb_p (4 pairs on DVE, pair 4 on
    GpSimd); per [128,512] psum bank (2 chunks stacked via
    tile_position): B1 bias matmul (start) + 5 pair matmuls
    (contraction 128 = 2 groups) -> relu -> x2dual.
  - L2 per block (=4-chunk group): t-scheme: pt2 [128,512] = 4x W2
    matmuls + 4x B2 selector matmuls; pcb3 = 4x S3 selector matmuls;
    Act-evict both to bf16; m2 = t*cb3 (DVE); R3 reduce -> pyT [12,512]
    psum -> DMA straight to DRAM (host un-shuffles the [12, 2048]
    feature-major output; pure indexing, no host flops).

DMA plan: sync queue carries xrep/corep slices FIRST, then cb tiles
[128, 2048] in block-major order (just-in-time stream, pool-throttled);
scalar queue carries the small blobs and the output tiles. All matmul
inputs bf16 (PE 1 col/cycle), fp32 PSUM accumulate.
"""
import sys

sys.path.insert(0, "/opt/trn_rl_repo")

import numpy as np

import concourse.bacc as bacc
import concourse.bass as bass
import concourse.mybir as mybir
import concourse.tile as tile
from concourse.bass_utils import run_bass_kernel_spmd

N_CORES = 8
B = 65536
G = 10
CI, H, CO = 3, 64, 3
B_LOC = B // N_CORES

F32 = mybir.dt.float32
BF16 = mybir.dt.bfloat16


def host_constants(W0, W1, W2, b0, b1, b2):
    """Pack constants into two bf16 blobs (cast on host; layout-only).

    blob42 [42, 256]: W0dup[0:128] | B1sel[128:192] | B2sel[192:224] | S3[224:256]
    blob128 [128, 396]: W1s[0:320] | W2lo[320:352] | W2hi[352:384] | R3[384:396]
    """
    import ml_dtypes
    blob42 = np.zeros((42, 384), np.float32)
    W0dup = blob42[:, 0:128]
    B1sel = blob42[:, 128:192]
    B2sel = blob42[:, 192:224]
    S3 = blob42[:, 224:256]
    sel2 = blob42[:, 256:384]      # rows 0:10 used: pair-4 cb broadcast
    sel2[8, 0:64] = 1.0
    sel2[9, 64:128] = 1.0
    for n in range(G):
        for i in range(CI):
            W0dup[n * 3 + i, 0:64] = W0[n, i]
            W0dup[n * 3 + i, 64:128] = W0[n, i]
        W0dup[32 + n, 0:64] = b0[n]
        W0dup[32 + n, 64:128] = b0[n]
        B1sel[32 + n] = b1[n]
        for o in range(CO):
            B2sel[32 + n, n * 3 + o] = b2[n, o]
            S3[32 + n, n * 3 + o] = 1.0
    blob128 = np.zeros((128, 396), np.float32)
    W1s = blob128[:, 0:320]
    W2lo = blob128[0:64, 320:352]
    W2hi = blob128[64:128, 352:384]
    R3 = blob128[:, 384:396]
    for p in range(5):
        W1s[:H, p * H:(p + 1) * H] = W1[2 * p]
        W1s[H:, p * H:(p + 1) * H] = W1[2 * p + 1]
    for n in range(G):
        for o in range(CO):
            W2lo[:, n * 3 + o] = W2[n, :, o]
            W2hi[:, n * 3 + o] = W2[n, :, o]
    for c in range(4):
        for n in range(G):
            for o in range(CO):
                R3[32 * c + n * 3 + o, c * 3 + o] = 1.0
    return dict(
        blob42=blob42.astype(ml_dtypes.bfloat16),
        blob128=blob128.astype(ml_dtypes.bfloat16),
    )


def make_reps(x_loc, co_loc, b_loc=B_LOC):
    """Host-side zero-flop replication: feature-major row-replicated x and co
    in u0coT row layout (rows n*3+i -> x_i / co_n; rows 32+n -> 1 / co_n)."""
    import ml_dtypes
    S = b_loc // 128
    xT = x_loc.reshape(128, S, CI).transpose(2, 1, 0).reshape(CI, b_loc)
    coT = co_loc.reshape(128, S, G).transpose(2, 1, 0).reshape(G, b_loc)
    xrep = np.zeros((42, b_loc), np.float32)
    corep = np.zeros((42, b_loc), np.float32)
    for n in range(G):
        for i in range(CI):
            xrep[n * 3 + i] = xT[i]
            corep[n * 3 + i] = coT[n]
        xrep[32 + n] = 1.0
        corep[32 + n] = coT[n]
    return xrep.astype(ml_dtypes.bfloat16), corep.astype(ml_dtypes.bfloat16)


def make_cbrep(co_loc, b_loc=B_LOC):
    """Host-side zero-flop layout prep: replicate co rows into the broadcast
    layout the kernel's multiply expects (bf16, u0coT column order
    col = s*128 + p <-> sample b = p*S + s)."""
    import ml_dtypes
    S = b_loc // 128
    arr = co_loc.astype(ml_dtypes.bfloat16)          # [b_loc, 10]
    coT = arr.reshape(128, S, G).transpose(2, 1, 0).reshape(G, b_loc)
    cb = np.empty((4, 128, b_loc), dtype=ml_dtypes.bfloat16)
    for p in range(4):
        cb[p, :64] = coT[2 * p]
        cb[p, 64:] = coT[2 * p + 1]
    return cb, np.ascontiguousarray(coT)


def build(nc, b_loc=B_LOC):
    CHUNKS = b_loc // 512      # 512-col chunks (16)
    BLOCKS = CHUNKS // 4       # 2048-col blocks (4); L2 groups == blocks
    DT = 2048

    xr_d = nc.declare_dram_parameter("xrep", [42, b_loc], BF16, isOutput=False)
    cor_d = nc.declare_dram_parameter("corep", [42, b_loc], BF16, isOutput=False)
    b42_d = nc.declare_dram_parameter("blob42", [42, 384], BF16, isOutput=False)
    b128_d = nc.declare_dram_parameter("blob128", [128, 396], BF16, isOutput=False)
    cb_d = nc.declare_dram_parameter("cbrep", [4, 128, b_loc], BF16, isOutput=False)
    coT_d = nc.declare_dram_parameter("coT", [10, b_loc], BF16, isOutput=False)
    out_d = nc.declare_dram_parameter("out", [12, b_loc // 4], F32, isOutput=True)

    with tile.TileContext(nc) as tc:
        with (
            tc.tile_pool(name="consts", bufs=1) as consts,
            tc.tile_pool(name="chain", bufs=1) as chain,
            tc.tile_pool(name="cbs", bufs=10) as cbs_pool,
            tc.tile_pool(name="xps", bufs=10) as xps_pool,
            tc.tile_pool(name="l2s", bufs=3) as l2s,
            tc.tile_pool(name="psA", bufs=4, space="PSUM") as psA,
            tc.tile_pool(name="psB", bufs=2, space="PSUM") as psB,
            tc.tile_pool(name="psCB", bufs=2, space="PSUM") as psCB,
        ):
            # ---- small loads on the scalar queue ----
            b42 = consts.tile([42, 384], BF16)
            nc.scalar.dma_start(b42[:], b42_d[:])
            b128 = consts.tile([128, 396], BF16)
            nc.scalar.dma_start(b128[:], b128_d[:])
            coT = consts.tile([10, b_loc], BF16)
            nc.sync.dma_start(coT[:], coT_d[:])  # head of sync queue: tiny
            W0dup = b42[:, 0:128]
            B1 = b42[:, 128:192]
            B2 = b42[:, 192:224]
            S3 = b42[:, 224:256]
            sel2 = b42[0:10, 256:384]
            W1s = b128[:, 0:320]
            W2lo = b128[:, 320:352]
            W2hi = b128[:, 352:384]
            R3 = b128[:, 384:396]

            # ---- xrep/corep interleaved with cb block 0 on the sync queue ----
            xrep = chain.tile([42, b_loc], BF16, tag="bigA")
            corep = chain.tile([42, b_loc], BF16, tag="bigB")
            cb_tiles = {}

            def load_cb(bblk, p):
                t = cbs_pool.tile([128, DT], BF16, tag="cb")
                nc.sync.dma_start(
                    t[:], cb_d[p, :, bblk * DT:(bblk + 1) * DT]
                )
                cb_tiles[(bblk, p)] = t

            for q in range(4):
                sl = slice(q * DT, (q + 1) * DT)
                nc.sync.dma_start(xrep[:, sl], xr_d[:, sl])
                nc.sync.dma_start(corep[:, sl], cor_d[:, sl])
            for bblk in range(BLOCKS):
                for p in range(4):
                    load_cb(bblk, p)

            # ---- pair-4 cb via PE selector broadcast from compact coT ----
            # evictions alternate Act / GpSimd to avoid overloading either
            cb4_sbs = []
            for bblk in range(BLOCKS):
                t = chain.tile([128, DT], BF16, tag=f"cb4_{bblk}")
                cb4_sbs.append(t)

            def emit_bc(c):
                pcb4 = psCB.tile([128, 512], F32, tag="cb4ps")
                nc.tensor.matmul(
                    pcb4[:], sel2[:], coT[:, c * 512:(c + 1) * 512]
                )
                dst = cb4_sbs[c // 4][:, (c % 4) * 512:(c % 4) * 512 + 512]
                if c % 2 == 0:
                    nc.scalar.activation(
                        dst, pcb4[:], mybir.ActivationFunctionType.Copy
                    )
                else:
                    nc.vector.tensor_copy(dst, pcb4[:])

            # ---- u0coT = xrep * corep (8 slices for early L0 start) ----
            u0coT = chain.tile([42, b_loc], BF16)
            for qq in range(8):
                sl = slice(qq * 1024, (qq + 1) * 1024)
                nc.vector.tensor_tensor(
                    out=u0coT[:, sl], in0=xrep[:, sl], in1=corep[:, sl],
                    op=mybir.AluOpType.mult,
                )

            # ---- L0: psum z1 duplicated via [W0flat | W0flat] stationary ----
            x1dup = chain.tile([128, b_loc], BF16)
            x2dual = chain.tile([128, b_loc // 2], BF16)
            for c in range(CHUNKS):
                pz = psA.tile([128, 512], F32, tag="z")
                nc.tensor.matmul(pz[:], W0dup[:], u0coT[:, c * 512:(c + 1) * 512])
                nc.scalar.activation(
                    x1dup[:, c * 512:(c + 1) * 512], pz[:],
                    mybir.ActivationFunctionType.Relu,
                )
                emit_bc(c)  # pair-4 cb broadcast woven between L0 chunks

            # ---- L1 + L2 per 2048-block ----
            def emit_l2(g):
                pt2 = psB.tile([128, 512], F32, tag="cbps")
                pcb3 = psB.tile([128, 512], F32, tag="cbps")
                for q in range(4):
                    c = 4 * g + q
                    d_abs, h = divmod(c, 2)
                    W2v = W2lo if h == 0 else W2hi
                    nc.tensor.matmul(
                        pt2[32 * q:32 * q + 32, :], W2v[:],
                        x2dual[:, d_abs * 512:(d_abs + 1) * 512],
                        tile_position=(0, 32 * q),
                        start=True, stop=False,
                        skip_group_check=True,
                    )
                for q in range(4):
                    c = 4 * g + q
                    nc.tensor.matmul(
                        pt2[32 * q:32 * q + 32, :], B2[:],
                        u0coT[:, c * 512:(c + 1) * 512],
                        tile_position=(0, 32 * q),
                        start=False, stop=True,
                        skip_group_check=True,
                    )
                for q in range(4):
                    c = 4 * g + q
                    nc.tensor.matmul(
                        pcb3[32 * q:32 * q + 32, :], S3[:],
                        u0coT[:, c * 512:(c + 1) * 512],
                        tile_position=(0, 32 * q),
                    )
                t2_sb = l2s.tile([128, 512], BF16, tag="t2_sb")
                nc.scalar.activation(
                    t2_sb[:], pt2[:], mybir.ActivationFunctionType.Copy
                )
                cb3_sb = l2s.tile([128, 512], BF16, tag="cb3_sb")
                nc.scalar.activation(
                    cb3_sb[:], pcb3[:], mybir.ActivationFunctionType.Copy
                )
                m2_sb = l2s.tile([128, 512], BF16, tag="m2_sb")
                nc.vector.tensor_tensor(
                    out=m2_sb[:], in0=t2_sb[:], in1=cb3_sb[:],
                    op=mybir.AluOpType.mult,
                )
                pyT = psA.tile([12, 512], F32, tag="z")
                nc.tensor.matmul(pyT[:], R3[:], m2_sb[:])
                yT_sb = l2s.tile([12, 512], F32, tag="yT_sb")
                nc.scalar.activation(
                    yT_sb[:], pyT[:], mybir.ActivationFunctionType.Copy
                )
                nc.scalar.dma_start(out_d[:, g * 512:(g + 1) * 512], yT_sb[:])

            for bblk in range(BLOCKS):
                bsl = slice(bblk * DT, (bblk + 1) * DT)
                # xp products, all on DVE (GpSimd TT measured ~5x slower)
                xps = []
                for p in range(5):
                    xp = xps_pool.tile([128, DT], BF16, tag="xp")
                    in1 = cb4_sbs[bblk] if p == 4 else cb_tiles[(bblk, p)]
                    nc.vector.tensor_tensor(
                        out=xp[:], in0=x1dup[:, bsl], in1=in1[:],
                        op=mybir.AluOpType.mult,
                    )
                    xps.append(xp)
                if bblk > 0:
                    emit_l2(bblk - 1)
                for dd in range(2):  # two chunk-pairs (psum banks) per block
                    pz2 = psA.tile([128, 512], F32, tag="z")
                    for h in range(2):
                        c = 4 * bblk + 2 * dd + h
                        nc.tensor.matmul(
                            pz2[64 * h:64 * h + 64, :], B1[:],
                            u0coT[:, c * 512:(c + 1) * 512],
                            tile_position=(0, 64 * h),
                            start=True, stop=False,
                            skip_group_check=True,
                        )
                    for p in range(5):
                        for h in range(2):
                            cc = 2 * dd + h
                            nc.tensor.matmul(
                                pz2[64 * h:64 * h + 64, :],
                                W1s[:, p * H:(p + 1) * H],
                                xps[p][:, cc * 512:(cc + 1) * 512],
                                tile_position=(0, 64 * h),
                                start=False, stop=(p == 4),
                                skip_group_check=True,
                            )
                    d_abs = 2 * bblk + dd
                    nc.scalar.activation(
                        x2dual[:, d_abs * 512:(d_abs + 1) * 512], pz2[:],
                        mybir.ActivationFunctionType.Relu,
                    )
            emit_l2(BLOCKS - 1)
    nc.compile()
    return nc


_NC_CACHE = {}


def get_nc(b_loc=B_LOC):
    if b_loc not in _NC_CACHE:
        nc = bacc.Bacc(None, target_bir_lowering=False)
        _NC_CACHE[b_loc] = build(nc, b_loc)
    return _NC_CACHE[b_loc]


def _unshuffle(yT, b_loc=B_LOC):
    """[12, b_loc/4] feature-major tiles -> [b_loc, 3] batch-major."""
    S = b_loc // 128
    y = np.empty((b_loc, CO), np.float32)
    r = np.arange(12)
    cq, o = r // 3, r % 3
    j = np.arange(512)
    for g in range(b_loc // DT_OUT):
        cg = (4 * g + cq[:, None]) * 512 + j[None, :]      # [12, 512] global col
        b_idx = (cg % 128) * S + cg // 128
        y[b_idx, np.broadcast_to(o[:, None], (12, 512))] = \
            yT[:, g * 512:(g + 1) * 512]
    return y


DT_OUT = 2048


def kernel(input, co_mat, W0, W1, W2, b0, b1, b2, _trace=False):
    input = np.asarray(input, np.float32)
    co_mat = np.asarray(co_mat, np.float32)
    consts = host_constants(
        np.asarray(W0, np.float32), np.asarray(W1, np.float32),
        np.asarray(W2, np.float32), np.asarray(b0, np.float32),
        np.asarray(b1, np.float32), np.asarray(b2, np.float32),
    )
    nc = get_nc()
    in_maps = []
    for k in range(N_CORES):
        sl = slice(k * B_LOC, (k + 1) * B_LOC)
        xr, cr = make_reps(input[sl], co_mat[sl])
        cb, coT = make_cbrep(co_mat[sl])
        m = {"xrep": xr, "corep": cr, "cbrep": cb, "coT": coT}
        m.update(consts)
        in_maps.append(m)
    res = run_bass_kernel_spmd(
        nc, in_maps, core_ids=list(range(N_CORES)), trace=_trace
    )
    out = np.concatenate(
        [_unshuffle(res.results[k]["out"]) for k in range(N_CORES)], axis=0
    )
    if _trace:
        kernel.last_exec_time_ns = res.exec_time_ns
    return out


kernel.last_exec_time_ns = None



# revision 2
# speedup vs baseline: 1.0426x; 1.0426x over previous
"""AdaptiveMLP Trainium2 kernel (8-core data parallel), v3.

Math per layer: y[b,o] = sum_{n,i} co[b,n]*x[b,i]*W[n,i,o] + sum_n co[b,n]*b[n,o]

Feature-major chain per core (B_LOC=8192 samples; column c <-> sample
b = (c%128)*64 + c//128):
  - u0coT [42, B]: rows n*3+i = co_n*x_i, rows 32+n = co_n (DVE mult of
    host-prepared xrep/corep).
  - L0: one matmul per 512-chunk with stationary [W0flat | W0flat]
    [42, 128] -> psum [128, 512] holds z1 TWICE (rows 0:64 == 64:128);
    single Act relu -> x1dup [128, B] (duplication for free, no DMA).
  - L1 per 2048-block: xp_p = x1dup * cb_p (5 pairs on DVE; ALL cb
    pairs now DMA-shipped, none PE-generated); per [128,512] psum bank
    (2 chunks stacked via tile_position): B1 bias matmul (start) + 5
    pair matmuls (contraction 128 = 2 groups) -> relu -> x2dual.
  - L2 per block (=4-chunk group): pt2 [128,512] = 4x W2 matmuls ONLY
    (rows 32q+3n+o = t_{n,o} of chunk 4g+q); Act Identity eviction adds
    the constant b2[n,o] per-partition bias for free -> t2b; m2 =
    t2b*cb3 (DVE; cb3 co-selector tile comes from host, replacing v2's
    16 S3 selector matmuls); R3 reduce -> pyT [12,512] psum -> DMA to
    DRAM (host un-shuffles; pure indexing, no host flops).

v3 vs v2: PE streams 180 -> 132 (B2+S3 selector matmuls replaced by
Act bias + tiny cb3 DMA; pair-4 cb broadcast moved from PE to DMA),
which also fixes v2's co^2*b2 bias quirk. DMA issue is split across
the sync and gpsimd queues with block-interleaved arrival order and
8KB descriptors ([128,4096] half-tiles).
"""
import sys

sys.path.insert(0, "/opt/trn_rl_repo")

import numpy as np

import concourse.bacc as bacc
import concourse.bass as bass
import concourse.mybir as mybir
import concourse.tile as tile
from concourse.bass_utils import run_bass_kernel_spmd

N_CORES = 8
B = 65536
G = 10
CI, H, CO = 3, 64, 3
B_LOC = B // N_CORES

F32 = mybir.dt.float32
BF16 = mybir.dt.bfloat16


def host_constants(W0, W1, W2, b0, b1, b2):
    """Pack constants into bf16 blobs (cast on host; layout-only).

    blob42 [42, 192]: W0dup[0:128] | B1sel[128:192]
    blob128 [128, 396]: W1s[0:320] | W2lo[320:352] | W2hi[352:384] | R3[384:396]
    b2vec [128, 1] f32: rows 32q+3n+o = b2[n,o] (Act eviction bias)
    """
    import ml_dtypes
    blob42 = np.zeros((42, 192), np.float32)
    W0dup = blob42[:, 0:128]
    B1sel = blob42[:, 128:192]
    for n in range(G):
        for i in range(CI):
            W0dup[n * 3 + i, 0:64] = W0[n, i]
            W0dup[n * 3 + i, 64:128] = W0[n, i]
        W0dup[32 + n, 0:64] = b0[n]
        W0dup[32 + n, 64:128] = b0[n]
        B1sel[32 + n] = b1[n]
    blob128 = np.zeros((128, 396), np.float32)
    W1s = blob128[:, 0:320]
    W2lo = blob128[0:64, 320:352]
    W2hi = blob128[64:128, 352:384]
    R3 = blob128[:, 384:396]
    for p in range(5):
        W1s[:H, p * H:(p + 1) * H] = W1[2 * p]
        W1s[H:, p * H:(p + 1) * H] = W1[2 * p + 1]
    for n in range(G):
        for o in range(CO):
            W2lo[:, n * 3 + o] = W2[n, :, o]
            W2hi[:, n * 3 + o] = W2[n, :, o]
    for c in range(4):
        for n in range(G):
            for o in range(CO):
                R3[32 * c + n * 3 + o, c * 3 + o] = 1.0
    b2vec = np.zeros((128, 1), np.float32)
    for q in range(4):
        for n in range(G):
            for o in range(CO):
                b2vec[32 * q + n * 3 + o, 0] = b2[n, o]
    return dict(
        blob42=blob42.astype(ml_dtypes.bfloat16),
        blob128=blob128.astype(ml_dtypes.bfloat16),
        b2vec=b2vec,
    )


def make_reps(x_loc, co_loc, b_loc=B_LOC):
    """Host-side zero-flop replication: feature-major row-replicated x and co
    in u0coT row layout (rows n*3+i -> x_i / co_n; rows 32+n -> 1 / co_n)."""
    import ml_dtypes
    S = b_loc // 128
    xT = x_loc.reshape(128, S, CI).transpose(2, 1, 0).reshape(CI, b_loc)
    coT = co_loc.reshape(128, S, G).transpose(2, 1, 0).reshape(G, b_loc)
    xrep = np.zeros((42, b_loc), np.float32)
    corep = np.zeros((42, b_loc), np.float32)
    for n in range(G):
        for i in range(CI):
            xrep[n * 3 + i] = xT[i]
            corep[n * 3 + i] = coT[n]
        xrep[32 + n] = 1.0
        corep[32 + n] = coT[n]
    return xrep.astype(ml_dtypes.bfloat16), corep.astype(ml_dtypes.bfloat16)


def make_cbs(co_loc, b_loc=B_LOC):
    """Host-side zero-flop layout prep (bf16 cast only):
    cb5 [5, 128, b_loc]: pair p rows 0:64 = co_{2p}, 64:128 = co_{2p+1}
      in u0coT column order (col = s*128 + p <-> sample b = p*S + s).
    cb3 [128, b_loc//4]: block g at cols g*512; rows 32q+3n+o = co_n of
      chunk 4g+q's columns (the L2 co-selector, replaces v2's S3 matmuls).
    """
    import ml_dtypes
    S = b_loc // 128
    arr = co_loc.astype(ml_dtypes.bfloat16)          # [b_loc, 10]
    coT = arr.reshape(128, S, G).transpose(2, 1, 0).reshape(G, b_loc)
    cb5 = np.empty((5, 128, b_loc), dtype=ml_dtypes.bfloat16)
    for p in range(5):
        cb5[p, :64] = coT[2 * p]
        cb5[p, 64:] = coT[2 * p + 1]
    cb3 = np.zeros((128, b_loc // 4), dtype=ml_dtypes.bfloat16)
    for g in range(b_loc // 2048):
        for q in range(4):
            c = 4 * g + q
            for n in range(G):
                row = 32 * q + 3 * n
                seg = coT[n, c * 512:(c + 1) * 512]
                for o in range(CO):
                    cb3[row + o, g * 512:(g + 1) * 512] = seg
    return cb5, cb3


def build(nc, b_loc=B_LOC):
    CHUNKS = b_loc // 512      # 512-col chunks (16)
    BLOCKS = CHUNKS // 4       # 2048-col blocks (4); L2 groups == blocks
    DT = 2048
    CBW = 4096                 # cb load half-tile width (8KB descriptors)

    xr_d = nc.declare_dram_parameter("xrep", [42, b_loc], BF16, isOutput=False)
    cor_d = nc.declare_dram_parameter("corep", [42, b_loc], BF16, isOutput=False)
    b42_d = nc.declare_dram_parameter("blob42", [42, 192], BF16, isOutput=False)
    b128_d = nc.declare_dram_parameter("blob128", [128, 396], BF16, isOutput=False)
    b2v_d = nc.declare_dram_parameter("b2vec", [128, 1], F32, isOutput=False)
    cb_d = nc.declare_dram_parameter("cb5", [5, 128, b_loc], BF16, isOutput=False)
    cb3_d = nc.declare_dram_parameter("cb3", [128, b_loc // 4], BF16, isOutput=False)
    out_d = nc.declare_dram_parameter("out", [12, b_loc // 4], F32, isOutput=True)

    with tile.TileContext(nc) as tc:
        with (
            tc.tile_pool(name="consts", bufs=1) as consts,
            tc.tile_pool(name="chain", bufs=1) as chain,
            tc.tile_pool(name="cbs", bufs=8) as cbs_pool,
            tc.tile_pool(name="xps", bufs=10) as xps_pool,
            tc.tile_pool(name="l2s", bufs=3) as l2s,
            tc.tile_pool(name="psZ", bufs=3, space="PSUM") as psZ,
            tc.tile_pool(name="psL1", bufs=3, space="PSUM") as psL1,
            tc.tile_pool(name="psT", bufs=2, space="PSUM") as psT,
        ):
            # ---- small loads on the scalar queue ----
            b42 = consts.tile([42, 192], BF16)
            nc.scalar.dma_start(b42[:], b42_d[:])
            b128 = consts.tile([128, 396], BF16)
            nc.scalar.dma_start(b128[:], b128_d[:])
            b2v = consts.tile([128, 1], F32)
            nc.scalar.dma_start(b2v[:], b2v_d[:])
            W0dup = b42[:, 0:128]
            B1 = b42[:, 128:192]
            W1s = b128[:, 0:320]
            W2lo = b128[:, 320:352]
            W2hi = b128[:, 352:384]
            R3 = b128[:, 384:396]

            # ---- big streams: sync queue (xrep/corep + cb pairs 0-2),
            #      gpsimd queue (cb pairs 3-4 + cb3), block-interleaved ----
            xrep = chain.tile([42, b_loc], BF16, tag="bigA")
            corep = chain.tile([42, b_loc], BF16, tag="bigB")
            cb_tiles = {}       # (p, half) -> [128, CBW] tile
            cb3 = chain.tile([128, b_loc // 4], BF16, tag="cb3")

            def load_cb(eng, p, h):
                t = cbs_pool.tile([128, CBW], BF16, tag="cb")
                eng.dma_start(t[:], cb_d[p, :, h * CBW:(h + 1) * CBW])
                cb_tiles[(p, h)] = t

            # sync queue issue order
            for q in range(2):
                sl = slice(q * DT, (q + 1) * DT)
                nc.sync.dma_start(xrep[:, sl], xr_d[:, sl])
                nc.sync.dma_start(corep[:, sl], cor_d[:, sl])
            for p in range(3):
                load_cb(nc.sync, p, 0)
            for q in range(2, 4):
                sl = slice(q * DT, (q + 1) * DT)
                nc.sync.dma_start(xrep[:, sl], xr_d[:, sl])
                nc.sync.dma_start(corep[:, sl], cor_d[:, sl])
            for p in range(3):
                load_cb(nc.sync, p, 1)

            # gpsimd queue: gate behind xrep slice 0 so the shared DMA
            # engines give the pipeline-critical head to the sync queue
            gate = consts.tile([1, 1], BF16)
            nc.gpsimd.tensor_copy(gate[:], xrep[0:1, 0:1])
            for p in (3, 4):
                load_cb(nc.gpsimd, p, 0)
            nc.gpsimd.dma_start(cb3[:], cb3_d[:])
            for p in (3, 4):
                load_cb(nc.gpsimd, p, 1)

            # ---- u0coT = xrep * corep (per-quarter, woven into L0) ----
            u0coT = chain.tile([42, b_loc], BF16)
            x1dup = chain.tile([128, b_loc], BF16)
            x2dual = chain.tile([128, b_loc // 2], BF16)

            for c in range(CHUNKS):
                if c % 4 == 0:
                    sl = slice(c * 512, c * 512 + DT)
                    nc.vector.tensor_tensor(
                        out=u0coT[:, sl], in0=xrep[:, sl], in1=corep[:, sl],
                        op=mybir.AluOpType.mult,
                    )
                pz = psZ.tile([128, 512], F32, tag="z")
                nc.tensor.matmul(pz[:], W0dup[:], u0coT[:, c * 512:(c + 1) * 512])
                nc.scalar.activation(
                    x1dup[:, c * 512:(c + 1) * 512], pz[:],
                    mybir.ActivationFunctionType.Relu,
                )

            # ---- L2 for one block: 4 W2 matmuls + bias-on-evict + m2 + R3 ----
            def emit_l2(g):
                pt2 = psT.tile([128, 512], F32, tag="t2")
                for q in range(4):
                    c = 4 * g + q
                    d_abs, h = divmod(c, 2)
                    W2v = W2lo if h == 0 else W2hi
                    nc.tensor.matmul(
                        pt2[32 * q:32 * q + 32, :], W2v[:],
                        x2dual[:, d_abs * 512:(d_abs + 1) * 512],
                        tile_position=(0, 32 * q),
                        skip_group_check=True,
                    )
                t2b = l2s.tile([128, 512], BF16, tag="t2b")
                nc.scalar.activation(
                    t2b[:], pt2[:], mybir.ActivationFunctionType.Identity,
                    bias=b2v[:, 0:1],
                )
                m2 = l2s.tile([128, 512], BF16, tag="m2")
                nc.vector.tensor_tensor(
                    out=m2[:], in0=t2b[:], in1=cb3[:, g * 512:(g + 1) * 512],
                    op=mybir.AluOpType.mult,
                )
                pyT = psZ.tile([12, 512], F32, tag="z")
                nc.tensor.matmul(pyT[:], R3[:], m2[:])
                yT_sb = l2s.tile([12, 512], F32, tag="yT")
                nc.scalar.activation(
                    yT_sb[:], pyT[:], mybir.ActivationFunctionType.Copy
                )
                nc.scalar.dma_start(out_d[:, g * 512:(g + 1) * 512], yT_sb[:])

            # ---- L1 + L2 per 2048-block ----
            for bblk in range(BLOCKS):
                bsl = slice(bblk * DT, (bblk + 1) * DT)
                xps = []
                for p in range(5):
                    xp = xps_pool.tile([128, DT], BF16, tag="xp")
                    cbt = cb_tiles[(p, bblk // 2)]
                    csl = slice((bblk % 2) * DT, (bblk % 2) * DT + DT)
                    nc.vector.tensor_tensor(
                        out=xp[:], in0=x1dup[:, bsl], in1=cbt[:, csl],
                        op=mybir.AluOpType.mult,
                    )
                    xps.append(xp)
                if bblk > 0:
                    emit_l2(bblk - 1)
                for dd in range(2):  # two chunk-pairs (psum banks) per block
                    pz2 = psL1.tile([128, 512], F32, tag="z2")
                    for h in range(2):
                        c = 4 * bblk + 2 * dd + h
                        nc.tensor.matmul(
                            pz2[64 * h:64 * h + 64, :], B1[:],
                            u0coT[:, c * 512:(c + 1) * 512],
                            tile_position=(0, 64 * h),
                            start=True, stop=False,
                            skip_group_check=True,
                        )
                    for p in range(5):
                        for h in range(2):
                            cc = 2 * dd + h
                            nc.tensor.matmul(
                                pz2[64 * h:64 * h + 64, :],
                                W1s[:, p * H:(p + 1) * H],
                                xps[p][:, cc * 512:(cc + 1) * 512],
                                tile_position=(0, 64 * h),
                                start=False, stop=(p == 4),
                                skip_group_check=True,
                            )
                    d_abs = 2 * bblk + dd
                    nc.scalar.activation(
                        x2dual[:, d_abs * 512:(d_abs + 1) * 512], pz2[:],
                        mybir.ActivationFunctionType.Relu,
                    )
            emit_l2(BLOCKS - 1)
    nc.compile()
    return nc


_NC_CACHE = {}


def get_nc(b_loc=B_LOC):
    if b_loc not in _NC_CACHE:
        nc = bacc.Bacc(None, target_bir_lowering=False)
        _NC_CACHE[b_loc] = build(nc, b_loc)
    return _NC_CACHE[b_loc]


def _unshuffle(yT, b_loc=B_LOC):
    """[12, b_loc/4] feature-major tiles -> [b_loc, 3] batch-major."""
    S = b_loc // 128
    y = np.empty((b_loc, CO), np.float32)
    r = np.arange(12)
    cq, o = r // 3, r % 3
    j = np.arange(512)
    for g in range(b_loc // DT_OUT):
        cg = (4 * g + cq[:, None]) * 512 + j[None, :]      # [12, 512] global col
        b_idx = (cg % 128) * S + cg // 128
        y[b_idx, np.broadcast_to(o[:, None], (12, 512))] = \
            yT[:, g * 512:(g + 1) * 512]
    return y


DT_OUT = 2048


def kernel(input, co_mat, W0, W1, W2, b0, b1, b2, _trace=False):
    input = np.asarray(input, np.float32)
    co_mat = np.asarray(co_mat, np.float32)
    consts = host_constants(
        np.asarray(W0, np.float32), np.asarray(W1, np.float32),
        np.asarray(W2, np.float32), np.asarray(b0, np.float32),
        np.asarray(b1, np.float32), np.asarray(b2, np.float32),
    )
    nc = get_nc()
    in_maps = []
    for k in range(N_CORES):
        sl = slice(k * B_LOC, (k + 1) * B_LOC)
        xr, cr = make_reps(input[sl], co_mat[sl])
        cb5, cb3 = make_cbs(co_mat[sl])
        m = {"xrep": xr, "corep": cr, "cb5": cb5, "cb3": cb3}
        m.update(consts)
        in_maps.append(m)
    res = run_bass_kernel_spmd(
        nc, in_maps, core_ids=list(range(N_CORES)), trace=_trace
    )
    out = np.concatenate(
        [_unshuffle(res.results[k]["out"]) for k in range(N_CORES)], axis=0
    )
    if _trace:
        kernel.last_exec_time_ns = res.exec_time_ns
    return out


kernel.last_exec_time_ns = None
